# revision 2
# baseline (speedup 1.0000x reference)
"""Trainium2 Bass kernel for a GRU decoder with Luong attention (v2).

Problem (hardcoded shapes): B=32, S=64, T=64, H=512, V=32000.
  out = log_softmax(decoder(inputs)) with shape [B, T, V] fp32.

Sharding: data-parallel over batch. Each of the 8 cores processes 4 batch
rows end-to-end. No collectives.

v2 changes vs baseline:
- fp8 W_out loaded ONCE and kept SBUF-resident (its pool is allocated after
  the P1 scratch frees, so it fits); pass A and pass B both read it.
- Gate psum split in two banks (rz / n) so tanh(rz) starts before the
  n-gate matmuls finish; gate DVE ops read psum directly (no ghn copy).
- bf16 for W_ih/W_cc/enc/gx/attention tensors (halves setup DMA + SBUF).
- Attention + W_cc tanh (hot) compressed to 2 emissions/step so pass A
  row-block 0 chunks start at step 36; one 512-col exp chunk rides after
  each later step's n-tanh in the ACT queue.
- exp row-sum accumulation moved off ACT to Pool (tensor_reduce of the
  bf16 exp dump).
- lse via bit-trick + 2 Newton iterations (exp only) - no Ln table swap.
- Tail: pass A (m=1) on ACT runs concurrently with pass B (m=0) recompute
  on PE + subtract on DVE + stores; pass B (m=1) subtract split ACT/DVE.
"""

from contextlib import ExitStack

import numpy as np
import ml_dtypes

import concourse.bacc as bacc
import concourse.bass as bass
import concourse.mybir as mybir
import concourse.tile as tile
from concourse.masks import make_identity

F32 = mybir.dt.float32
BF16 = mybir.dt.bfloat16
FP8 = mybir.dt.float8e4
I32 = mybir.dt.int32
AF = mybir.ActivationFunctionType
ALU = mybir.AluOpType
AX = mybir.AxisListType
F32R = mybir.dt.float32r
DR = mybir.MatmulPerfMode.DoubleRow


def rr(ap):
    return ap.bitcast(F32R)


B, S, T, H, V = 32, 64, 64, 512, 32000
NC = 8
BL = B // NC          # 4 local batch rows
R = T * BL            # 256 local output rows, r = t*BL + b
NEG = -1e30
WS = 8.0              # fp8 weight scale for the output projection
NPAIR = 32            # weight pair-chunks of 1024 cols; last is 256 wide
PAIRW = [1024] * (NPAIR - 1) + [256]
PAIROFF = [1024 * i for i in range(NPAIR)]

import os
A0_START = int(os.environ.get("K_A0", 40))  # first in-loop pass-A step
CW_IL = int(os.environ.get("K_CW_IL", 512))    # in-loop chunk width
IL_EVERY = int(os.environ.get("K_IL_EVERY", 1))  # steps between in-loop chunks
OST_BUFS = int(os.environ.get("K_OST_BUFS", 4))
IL2_START = int(os.environ.get("K_IL2_START", 42))  # 2nd in-loop chunk from here
N_IL = ((T - A0_START + IL_EVERY - 1) // IL_EVERY +
        max(0, T - IL2_START)) if CW_IL else 0
N_IL = min(N_IL, (V + CW_IL - 1) // CW_IL) if CW_IL else 0
BW = int(os.environ.get("K_BW", 1024))              # pass-B chunk width
PSB_BUFS = int(os.environ.get("K_PSB_BUFS", 4))


def b_chunks():
    ch = []
    c = 0
    while c < V:
        w = min(BW, V - c)
        ch.append((c, w))
        c += w
    return ch


CHB = b_chunks()
NCHB = len(CHB)


def chunk_list(n_il):
    """m0: n_il in-loop CW_IL-col chunks, then 2048-col chunks (+odd tail)."""
    ch0 = []
    c = 0
    for _ in range(n_il):
        if c >= V:
            break
        w = min(CW_IL, V - c)
        ch0.append((c, w))
        c += w
    while c < V:
        w = min(1024, V - c)
        ch0.append((c, w))
        c += w
    ch1 = []
    c = 0
    while c < V:
        w = min(1024, V - c)
        ch1.append((c, w))
        c += w
    return ch0, ch1


CH0, CH1 = chunk_list(N_IL)
NCH0, NCH1 = len(CH0), len(CH1)
# Newton-lse constants: y0 = float(bits(R)) * LN2P23 - Y0B  ~=  ln(R)
LN2P23 = float(np.log(2.0) / (1 << 23))
Y0B = float((127.0 - 0.0430) * np.log(2.0))


def build_program():
    nc = bacc.Bacc(None, target_bir_lowering=False, debug=False)

    # ---- DRAM parameters (per-core slices prepared on host) ----
    emb_d = nc.declare_dram_parameter("emb", [V, H], F32, isOutput=False)
    ids_d = nc.declare_dram_parameter("ids", [2, 128, 1], I32, isOutput=False)
    h0T_d = nc.declare_dram_parameter("h0T", [128, 16], F32, isOutput=False)
    encT_d = nc.declare_dram_parameter("encT", [H, BL * S], BF16, isOutput=False)
    encS_d = nc.declare_dram_parameter("encS", [S, BL * H], BF16, isOutput=False)
    maskb_d = nc.declare_dram_parameter("maskb", [1, BL * S], F32, isOutput=False)
    actmT_d = nc.declare_dram_parameter("actmT", [128, T * 16], F32, isOutput=False)
    wihT_d = nc.declare_dram_parameter("wihT", [H, 3 * H], BF16, isOutput=False)
    whhT_d = nc.declare_dram_parameter("whhT", [H, 3 * H], BF16, isOutput=False)
    brow_d = nc.declare_dram_parameter("brow", [1, 3 * H], F32, isOutput=False)
    bhhn4_d = nc.declare_dram_parameter("bhhn4", [4, 128], F32, isOutput=False)
    sel4_d = nc.declare_dram_parameter("sel4", [4, 16], F32, isOutput=False)
    wccT_d = nc.declare_dram_parameter("wccT", [2 * H, H], BF16, isOutput=False)
    bcc_d = nc.declare_dram_parameter("bcc", [128, 4], F32, isOutput=False)
    # fp8 weight bytes disguised as int32 for the PJRT interface
    wo8_d = nc.declare_dram_parameter("wo8", [2, 128, V // 2], I32, isOutput=False)
    ones_d = nc.declare_dram_parameter("onesd", [1, 256], F32, isOutput=False)
    out_d = nc.declare_dram_parameter("out", [R, V], BF16, isOutput=True)

    with tile.TileContext(nc) as tc, ExitStack() as stk:
        constp = stk.enter_context(tc.tile_pool(name="const", bufs=1))
        histp = stk.enter_context(tc.tile_pool(name="hist", bufs=1))
        hotp = stk.enter_context(tc.tile_pool(name="hot", bufs=1))
        sump = stk.enter_context(tc.tile_pool(name="sums", bufs=1))
        dmp = stk.enter_context(tc.tile_pool(name="dump", bufs=2))
        ostp = stk.enter_context(tc.tile_pool(name="ost", bufs=OST_BUFS))

        # ---- constants / small tiles (SP queue) ----
        ident = constp.tile([128, 128], F32, tag="ident")
        make_identity(nc, ident[:])
        identb = constp.tile([128, 128], BF16, tag="identb")
        nc.vector.tensor_copy(identb[:], ident[:])
        ones256 = constp.tile([1, 256], F32, tag="ones256")
        nc.sync.dma_start(rr(ones256[:]), rr(ones_d[:]))
        maskb = constp.tile([1, BL * S], F32, tag="maskb")
        nc.sync.dma_start(rr(maskb[:]), rr(maskb_d[:]))
        actmT = constp.tile([128, T * 16], F32, tag="actmT")
        nc.sync.dma_start(actmT[:], actmT_d[:])
        bcc = constp.tile([128, 4], F32, tag="bcc")
        nc.sync.dma_start(bcc[:], bcc_d[:])
        bhhn4 = constp.tile([4, 128], F32, tag="bhhn4")
        nc.sync.dma_start(rr(bhhn4[:]), rr(bhhn4_d[:]))
        sel4 = constp.tile([4, 16], F32, tag="sel4")
        nc.sync.dma_start(rr(sel4[:]), rr(sel4_d[:]))

        # history tiles: col = t*16 + q*4 + b   (bf16)
        hnewT = histp.tile([128, T * 16], BF16, tag="hnewT")
        ctxT = histp.tile([128, T * 16], BF16, tag="ctxT")
        # hot in fp8, DoubleRow layout: hotA covers h-chunks (0,1),
        # hotB (2,3); col = k*R + r
        hotA = hotp.tile([128, 2 * R], FP8, tag="hotA")
        hotB = hotp.tile([128, 2 * R], FP8, tag="hotB")
        sets = [sump.tile([128, NCH0], F32, tag="sets0", name="sets0"),
                sump.tile([128, NCH1], F32, tag="sets1", name="sets1")]
        yls = [None, None]   # -lse tiles, written by emit_lse

        wo = [None] * NPAIR  # resident fp8 weight tiles, filled later
        hots = (hotA, hotB)

        # ---------- projection helpers ----------
        def mm_cols(ps, m, c0, w):
            """Accumulate logits for rows m*128.. cols [c0,c0+w) into ps."""
            done = 0
            while done < w:
                c = c0 + done
                sw = min(512 - (c % 512), w - done)
                jp, off = c // 1024, c % 1024
                dst = ps[:, done:done + sw]
                for p in range(2):
                    wpair = wo[jp][p]
                    rhs = wpair[:, 0:2 * PAIRW[jp]].rearrange(
                        "p (k n) -> p k n", k=2)[:, :, off:off + sw]
                    lhsT = hots[p][:].rearrange("p (k r) -> p k r", k=2)[
                        :, :, m * 128:(m + 1) * 128]
                    nc.tensor.matmul(dst, lhsT, rhs, start=(p == 0),
                                     stop=(p == 1), perf_mode=DR)
                done += sw

        def emit_A(m, ci, c0, w, pspool, pw, dve_sum=False):
            ps = pspool.tile([128, pw], F32, tag="A", name=f"psA{m}_{ci}")
            mm_cols(ps, m, c0, w)
            dump = dmp.tile([128, pw], BF16, tag=f"dump{pw}",
                            bufs=4 if pw <= 512 else 2, name=f"dm{m}_{ci}")
            if dve_sum:
                nc.scalar.activation(dump[:, 0:w], ps[:, 0:w], AF.Exp,
                                     scale=float(1.0 / WS))
                nc.vector.tensor_reduce(sets[m][:, ci:ci + 1], dump[:, 0:w],
                                        AX.X, ALU.add)
            else:
                nc.scalar.activation(dump[:, 0:w], ps[:, 0:w], AF.Exp,
                                     scale=float(1.0 / WS),
                                     accum_out=sets[m][:, ci:ci + 1])

        def emit_B(ps_pool, m, ci, c0, w, sub_act, st_eng):
            ps = ps_pool.tile([128, BW], F32, tag="B", name=f"psB{m}_{ci}")
            mm_cols(ps, m, c0, w)
            ost = ostp.tile([128, 1024], BF16, tag="ost", name=f"os{m}_{ci}")
            if sub_act:
                nc.scalar.activation(ost[:, 0:w], ps[:, 0:w], AF.Identity,
                                     scale=float(1.0 / WS),
                                     bias=yls[m][:, 0:1])
            else:
                nc.vector.tensor_scalar(ost[:, 0:w], ps[:, 0:w],
                                        float(1.0 / WS), yls[m][:, 0:1],
                                        ALU.mult, ALU.add)
            st_eng.dma_start(
                out_d[m * 128:(m + 1) * 128, c0:c0 + w], ost[:, 0:w])

        def emit_lse(m, nch):
            """yls[m] = -lse = ln(1/sum) via exp-only Newton (no Ln table)."""
            stot = sump.tile([128, 1], F32, tag=f"st{m}", name=f"stot{m}")
            nc.vector.tensor_reduce(stot[:], sets[m][:, 0:nch], AX.X, ALU.add)
            rec = sump.tile([128, 1], F32, tag=f"rc{m}", name=f"rec{m}")
            nc.vector.reciprocal(rec[:], stot[:])
            rf = sump.tile([128, 1], F32, tag=f"rf{m}", name=f"rf{m}")
            nc.vector.tensor_copy(rf[:], rec[:].bitcast(I32))
            y = sump.tile([128, 1], F32, tag=f"yini{m}", name=f"y0_{m}")
            nc.vector.tensor_scalar(y[:], rf[:], LN2P23, -Y0B,
                                    ALU.mult, ALU.add)
            for it in range(2):
                e = sump.tile([128, 1], F32, tag=f"e{m}{it}", name=f"e{m}_{it}")
                nc.scalar.activation(e[:], y[:], AF.Exp, scale=-1.0)
                tk = sump.tile([128, 1], F32, tag=f"t{m}{it}", name=f"t{m}_{it}")
                nc.vector.tensor_tensor(tk[:], rec[:], e[:], ALU.mult)
                y2 = sump.tile([128, 1], F32, tag=f"yit{m}_{it}", name=f"y{m}_{it}")
                nc.vector.scalar_tensor_tensor(y2[:], tk[:], -1.0, y[:],
                                               ALU.add, ALU.add)
                y = y2
            yls[m] = y

        with ExitStack() as stk1:
            wp = stk1.enter_context(tc.tile_pool(name="weights", bufs=1))
            hTp = stk1.enter_context(tc.tile_pool(name="hT", bufs=2))
            gp = stk1.enter_context(tc.tile_pool(name="gates", bufs=2))
            attp = stk1.enter_context(tc.tile_pool(name="att", bufs=2))
            gxp = stk1.enter_context(tc.tile_pool(name="gx", bufs=1))
            ps_rz = stk1.enter_context(tc.tile_pool(name="ps_rz", bufs=2,
                                                    space="PSUM"))
            ps_n = stk1.enter_context(tc.tile_pool(name="ps_n", bufs=2,
                                                   space="PSUM"))
            ps_a = stk1.enter_context(tc.tile_pool(name="ps_a", bufs=2,
                                                   space="PSUM"))
            ps_il = stk1.enter_context(tc.tile_pool(name="ps_il", bufs=2,
                                                    space="PSUM"))

            # ---- P1-transient pool (freed before the resident weights) ----
            stkp1 = ExitStack()
            p1p = stkp1.enter_context(tc.tile_pool(name="p1", bufs=1))
            # ids on SP, then embedding gathers on Pool queue
            ids_t = [None, None]
            for m in range(2):
                ids_t[m] = p1p.tile([128, 1], I32, tag=f"ids{m}", name=f"ids{m}")
                nc.sync.dma_start(ids_t[m][:], ids_d[m])
            xs_t = [None, None]
            for m in range(2):
                xs_t[m] = p1p.tile([128, H], F32, tag=f"xs{m}", name=f"xs{m}")
                nc.gpsimd.indirect_dma_start(
                    out=xs_t[m][:],
                    out_offset=None,
                    in_=emb_d[:],
                    in_offset=bass.IndirectOffsetOnAxis(ap=ids_t[m][:, 0:1],
                                                        axis=0),
                )
            # ---- loop-critical weights: whh on Pool, wih on SP ----
            whh = []
            for q in range(4):
                ht_ = wp.tile([128, 3 * H], BF16, tag=f"whh{q}")
                nc.gpsimd.dma_start(ht_[:], whhT_d[q * 128:(q + 1) * 128, :])
                whh.append(ht_)
            wih = []
            for q in range(4):
                wt_ = p1p.tile([128, 3 * H], BF16, tag=f"wih{q}")
                nc.sync.dma_start(wt_[:], wihT_d[q * 128:(q + 1) * 128, :])
                wih.append(wt_)
            brow = p1p.tile([1, 3 * H], F32, tag="brow")
            nc.sync.dma_start(rr(brow[:]), rr(brow_d[:]))
            h0T = hTp.tile([128, 16], F32, tag="hT", name="hT_init")
            nc.sync.dma_start(rr(h0T[:]), rr(h0T_d[:]))
            h0Tb = hTp.tile([128, 16], BF16, tag="hTb", name="hTb_init")
            nc.vector.tensor_copy(h0Tb[:], h0T[:])
            # ---- attention-path weights on Pool queue ----
            wcc = []
            for kt in range(8):
                w_ = wp.tile([128, H], BF16, tag=f"wcc{kt}")
                nc.gpsimd.dma_start(w_[:], wccT_d[kt * 128:(kt + 1) * 128, :])
                wcc.append(w_)
            encT = []
            for q in range(4):
                e_ = wp.tile([128, BL * S], BF16, tag=f"encT{q}")
                nc.gpsimd.dma_start(e_[:], encT_d[q * 128:(q + 1) * 128, :])
                encT.append(e_)
            encS = wp.tile([S, BL * H], BF16, tag="encS")
            nc.gpsimd.dma_start(encS[:], encS_d[:])

            # ---- P1: xsT transposes + gxT = (x @ W_ih.T + b).T ----
            gxT_rz = gxp.tile([128, T * 32], BF16, tag="gxrz")
            gxT_n = gxp.tile([128, T * 16], BF16, tag="gxn")
            with stkp1:
                xsT = [p1p.tile([128, 256], BF16, tag=f"xsT{q}", name=f"xsT{q}")
                       for q in range(4)]
                for m in range(2):
                    for q in range(4):
                        tp = ps_a.tile([128, 128], F32, tag="A", name=f"tp{m}_{q}")
                        nc.tensor.transpose(tp[:], xs_t[m][:, q * 128:(q + 1) * 128],
                                            ident[:])
                        nc.vector.tensor_copy(xsT[q][:, m * 128:(m + 1) * 128],
                                              tp[:])
                vrz = gxT_rz[:].rearrange("p (t j x) -> p t j x", j=2, x=16)
                vn = gxT_n[:].rearrange("p (t x) -> p t x", x=16)
                for j in range(3):
                    for ms in range(4):
                        col = j * 512 + ms * 128
                        gps = ps_a.tile([128, 256], F32, tag="A", name=f"gx{j}_{ms}")
                        for q in range(4):
                            nc.tensor.matmul(
                                gps[:], wih[q][:, col:col + 128], xsT[q][:],
                                start=(q == 0), stop=False,
                            )
                        nc.tensor.matmul(
                            gps[:], rr(brow[0:1, col:col + 128]),
                            rr(ones256[0:1, :]), start=False, stop=True,
                        )
                        src = gps[:].rearrange("p (t b) -> p t b", b=4)
                        if j < 2:
                            dst = vrz[:, :, j, ms * 4:(ms + 1) * 4]
                        else:
                            dst = vn[:, :, ms * 4:(ms + 1) * 4]
                        nc.vector.tensor_copy(dst, src)

            # ---- resident fp8 output weights (pool alloc after P1 frees) ----
            wop = stk.enter_context(tc.tile_pool(name="wo", bufs=1, side="right"))
            for jp in range(NPAIR):
                pair = []
                for p in range(2):
                    wt = wop.tile([128, 2 * PAIRW[jp]], FP8, tag=f"wo{jp}_{p}",
                                  name=f"wo{jp}_{p}")
                    o4 = PAIROFF[jp] // 2
                    nc.sync.dma_start(
                        wt[:].bitcast(I32),
                        wo8_d[p][:, o4:o4 + PAIRW[jp] // 2])
                    pair.append(wt)
                wo[jp] = pair

            # ---- attention / hot emitters ----
            def emit_attention(blk, b):
                c0 = blk * 256
                sc = ps_a.tile([16, S], F32, tag="A", name=f"sc{blk}_{b}")
                for q in range(4):
                    nc.tensor.matmul(
                        sc[:],
                        hnewT[:, c0 + q * 4 + b:c0 + 256:16],
                        encT[q][:, b * S:(b + 1) * S],
                        start=(q == 0), stop=False,
                    )
                nc.tensor.matmul(
                    sc[:], rr(ones256[0:1, 0:16]), rr(maskb[0:1, b * S:(b + 1) * S]),
                    start=False, stop=True,
                )
                se = attp.tile([16, 1], F32, tag="se", name=f"se{blk}_{b}")
                al = attp.tile([16, S], F32, tag="al", name=f"al{blk}_{b}")
                nc.scalar.activation(al[:], sc[:], AF.Exp)
                nc.vector.tensor_reduce(se[:], al[:], AX.X, ALU.add)
                rec = attp.tile([16, 1], F32, tag="rec", name=f"rc{blk}_{b}")
                nc.vector.reciprocal(rec[:], se[:])
                aln = attp.tile([16, S], F32, tag="aln", name=f"an{blk}_{b}")
                nc.vector.tensor_scalar_mul(aln[:], al[:], rec[:, 0:1])
                alT_ps = ps_a.tile([S, 16], F32, tag="A", name=f"tpa{blk}_{b}")
                nc.tensor.transpose(alT_ps[:], aln[:], ident[0:16, 0:16])
                alT = attp.tile([S, 16], BF16, tag="alT", name=f"at{blk}_{b}")
                nc.vector.tensor_copy(alT[:], alT_ps[:])
                cx = ps_a.tile([128, 64], F32, tag="A", name=f"cx{blk}_{b}")
                for q in range(4):
                    nc.tensor.matmul(
                        cx[:, q * 16:(q + 1) * 16],
                        encS[0:S, b * H + q * 128:b * H + (q + 1) * 128],
                        alT[:],
                        start=(q == 0), stop=(q == 3),
                    )
                csrc = cx[:].rearrange("p (q t) -> p q t", q=4)
                cdst = ctxT[:].rearrange("p (t q x) -> p q t x", q=4, x=4)[
                    :, :, blk * 16:(blk + 1) * 16, b
                ]
                nc.vector.tensor_copy(cdst, csrc)

            def emit_hot(blk, mh):
                hps = ps_a.tile([128, 64], F32, tag="A", name=f"hp{blk}_{mh}")
                for kt in range(8):
                    srcT = ctxT if kt < 4 else hnewT
                    q = kt % 4
                    rhs = srcT[:].rearrange("p (t x) -> p t x", x=16)[
                        :, blk * 16:(blk + 1) * 16, q * 4:(q + 1) * 4
                    ]
                    nc.tensor.matmul(
                        hps[:], wcc[kt][:, mh * 128:(mh + 1) * 128], rhs,
                        start=(kt == 0), stop=(kt == 7),
                    )
                dsttile = hotA if mh < 2 else hotB
                k = mh % 2
                nc.scalar.activation(
                    dsttile[:, k * R + blk * 64:k * R + (blk + 1) * 64],
                    hps[:], AF.Tanh, bias=bcc[:, mh:mh + 1],
                )

            # ---- P2: GRU recurrence ----
            hT = h0T
            hTb = h0Tb
            a0_idx = 0
            for t in range(T):
                prz = ps_rz.tile([128, 32], F32, tag="rz", name=f"rz{t}")
                first = True
                for j in range(2):
                    for ms in range(4):
                        col = j * 512 + ms * 128
                        dst = prz[:, j * 16 + ms * 4:j * 16 + (ms + 1) * 4]
                        for q in range(4):
                            nc.tensor.matmul(
                                dst, whh[q][:, col:col + 128],
                                hTb[:, q * 4:(q + 1) * 4],
                                start=first, stop=False,
                            )
                            first = False
                nc.tensor.matmul(
                    prz[:, 0:32], identb[:], gxT_rz[:, t * 32:(t + 1) * 32],
                    start=False, stop=True,
                )
                pn = ps_n.tile([128, 16], F32, tag="n", name=f"pn{t}")
                firstn = True
                for ms in range(4):
                    col = 2 * 512 + ms * 128
                    dst = pn[:, ms * 4:(ms + 1) * 4]
                    for q in range(4):
                        nc.tensor.matmul(
                            dst, whh[q][:, col:col + 128],
                            hTb[:, q * 4:(q + 1) * 4],
                            start=firstn, stop=False,
                        )
                        firstn = False
                nc.tensor.matmul(
                    pn[:, 0:16], rr(bhhn4[:]), rr(sel4[:]),
                    start=False, stop=True,
                )

                th = gp.tile([128, 32], F32, tag="th", name=f"th{t}")
                nc.scalar.activation(th[:], prz[:], AF.Tanh, scale=0.5)
                # a1 = (th_r + 1) * ghn in one DVE op (ghn read from psum)
                a1 = gp.tile([128, 16], F32, tag="a1", name=f"a1{t}")
                nc.vector.scalar_tensor_tensor(a1[:], th[:, 0:16], 1.0,
                                               pn[:, 0:16], ALU.add, ALU.mult)
                a2 = gp.tile([128, 16], F32, tag="a2", name=f"a2{t}")
                nc.vector.tensor_tensor(a2[:], a1[:], gxT_n[:, t * 16:(t + 1) * 16],
                                        ALU.add)
                n_ = gp.tile([128, 16], F32, tag="n", name=f"n{t}")
                nc.scalar.activation(n_[:], a2[:], AF.Tanh, scale=0.5)
                # gate blends prepared on Pool while the n-branch runs:
                # h2 = g_*n + (1-g_)*h ; hnew = g1*n + (1-g1)*h
                g1 = gp.tile([128, 16], F32, tag="g1", name=f"g1{t}")
                nc.gpsimd.tensor_scalar(g1[:], th[:, 16:32], -0.5, 0.5,
                                        ALU.mult, ALU.add)
                g_ = gp.tile([128, 16], F32, tag="g", name=f"g{t}")
                nc.gpsimd.tensor_tensor(g_[:], g1[:], actmT[:, t * 16:(t + 1) * 16],
                                        ALU.mult)
                # t2 = (g_ - 1)*h = -(1-g_)*h ; later h2 = t1 - t2
                t2 = gp.tile([128, 16], F32, tag="t2", name=f"t2{t}")
                nc.gpsimd.scalar_tensor_tensor(t2[:], g_[:], 1.0, hT[:],
                                               ALU.subtract, ALU.mult)
                t2n = gp.tile([128, 16], F32, tag="t2n", name=f"t2n{t}")
                nc.gpsimd.scalar_tensor_tensor(t2n[:], g1[:], 1.0, hT[:],
                                               ALU.subtract, ALU.mult)
                # t1 and the bf16 twin of h2 on Pool: PE unblocks off Pool
                t1 = gp.tile([128, 16], F32, tag="t1", name=f"t1{t}")
                nc.gpsimd.tensor_tensor(t1[:], g_[:], n_[:], ALU.mult)
                hT2b = hTp.tile([128, 16], BF16, tag="hTb", name=f"hTb{t}")
                nc.gpsimd.tensor_tensor(hT2b[:], t1[:], t2[:], ALU.subtract)
                hT2 = hTp.tile([128, 16], F32, tag="hT", name=f"hT{t}")
                nc.vector.tensor_tensor(rr(hT2[:]), t1[:], t2[:], ALU.subtract)
                # hnewT (unmasked h_new) off the critical path, on Pool
                t1n = gp.tile([128, 16], F32, tag="t1n", name=f"t1n{t}")
                nc.gpsimd.tensor_tensor(t1n[:], g1[:], n_[:], ALU.mult)
                nc.gpsimd.tensor_tensor(hnewT[:, t * 16:(t + 1) * 16], t1n[:],
                                        t2n[:], ALU.subtract)
                hT = hT2
                hTb = hT2b

                # attention/hot for block blk spread over block blk+1 steps
                if t >= 16:
                    blk, r_ = t // 16 - 1, t % 16
                    if r_ < 4:
                        emit_attention(blk, r_)
                    elif r_ < 8:
                        emit_hot(blk, r_ - 4)
                # overlapped pass-A chunks for row-block m=0
                if (t >= A0_START and a0_idx < N_IL
                        and (t - A0_START) % IL_EVERY == 0):
                    emit_A(0, a0_idx, CH0[a0_idx][0], CH0[a0_idx][1],
                           ps_il, CW_IL, dve_sum=False)
                    a0_idx += 1
                if t >= IL2_START and a0_idx < N_IL:
                    emit_A(0, a0_idx, CH0[a0_idx][0], CH0[a0_idx][1],
                           ps_il, CW_IL, dve_sum=False)
                    a0_idx += 1

            for b in range(4):
                emit_attention(3, b)
            for mh in range(4):
                emit_hot(3, mh)

        # ---- tail: rest of pass A, lse, pass B ----
        with tc.tile_pool(name="ps_A2", bufs=2, space="PSUM") as ps_A2, \
             tc.tile_pool(name="ps_B", bufs=2, space="PSUM") as ps_B:
            for ci in range(a0_idx, NCH0):
                emit_A(0, ci, CH0[ci][0], CH0[ci][1], ps_A2, 1024)
            emit_lse(0, NCH0)
            # pass A (m=1) on ACT concurrent with pass B (m=0) on PE/DVE/DMA
            bi = 0
            for i in range(NCH1):
                emit_A(1, i, CH1[i][0], CH1[i][1], ps_A2, 1024)
                while bi * NCH1 < (i + 1) * NCHB and bi < NCHB:
                    emit_B(ps_B, 0, bi, CHB[bi][0], CHB[bi][1], False,
                           nc.sync if bi % 2 == 0 else nc.gpsimd)
                    bi += 1
            emit_lse(1, NCH1)
        with tc.tile_pool(name="ps_B1", bufs=PSB_BUFS, space="PSUM") as ps_B1:
            for i in range(NCHB):
                emit_B(ps_B1, 1, i, CHB[i][0], CHB[i][1], i % 2 == 1,
                       nc.sync if i % 2 == 0 else nc.gpsimd)

    nc.compile()
    return nc


_NC_CACHE = None


def _get_program():
    global _NC_CACHE
    if _NC_CACHE is None:
        _NC_CACHE = build_program()
    return _NC_CACHE


def make_core_inputs(all_encoder_hidden_states, initial_decoder_hidden_state,
                     encoder_output_mask, target_input, fra_length, embedding,
                     W_ih, W_hh, b_ih, b_hh, W_cc, b_cc, W_out, b_out):
    """Build the per-core input maps (host-side sharding/layout only)."""
    f8 = ml_dtypes.float8_e4m3
    bf = ml_dtypes.bfloat16
    enc = np.ascontiguousarray(np.asarray(all_encoder_hidden_states, np.float32))
    h0 = np.asarray(initial_decoder_hidden_state, np.float32)[0]
    mask = np.asarray(encoder_output_mask)
    tgt = np.asarray(target_input).astype(np.int64)
    fra = np.asarray(fra_length).astype(np.int64)
    emb = np.ascontiguousarray(np.asarray(embedding, np.float32))
    W_ih = np.asarray(W_ih, np.float32)
    W_hh = np.asarray(W_hh, np.float32)
    b_ih = np.asarray(b_ih, np.float32)
    b_hh = np.asarray(b_hh, np.float32)

    wih_mod = W_ih.copy()
    wih_mod[2 * H:3 * H, :] *= 2.0          # n-gate pre-scaled by 2
    wihT = np.ascontiguousarray(wih_mod.T.astype(bf))
    whhT = np.ascontiguousarray(W_hh.T.astype(bf))
    brow = np.concatenate([
        (b_ih[:2 * H] + b_hh[:2 * H]),      # r,z: both biases, fold into gx
        2.0 * b_ih[2 * H:],                 # n: only b_ih (scaled)
    ])[None, :].astype(np.float32)
    bhhn4 = np.ascontiguousarray(b_hh[2 * H:].reshape(4, 128))
    sel4 = np.ascontiguousarray(np.repeat(np.eye(4, dtype=np.float32), 4, axis=1))
    wccT = np.ascontiguousarray(np.asarray(W_cc, np.float32).T.astype(bf))
    bcc4 = np.ascontiguousarray(np.asarray(b_cc, np.float32).reshape(4, 128).T)

    # fp8 output weights, DoubleRow pair layout
    w8 = (np.asarray(W_out, np.float32).T * WS).astype(f8)   # [H, V]
    wt4 = w8.reshape(4, 128, V)
    wo8 = np.empty((2, 128, 2 * V), f8)
    for p in range(2):
        blocks = []
        for jp in range(NPAIR):
            sl = slice(PAIROFF[jp], PAIROFF[jp] + PAIRW[jp])
            blocks.append(np.concatenate([wt4[2 * p][:, sl], wt4[2 * p + 1][:, sl]],
                                         axis=1))
        wo8[p] = np.concatenate(blocks, axis=1)
    wo8i = np.ascontiguousarray(wo8).view(np.uint8).astype(np.uint8).view(np.int32).reshape(2, 128, V // 2)

    in_maps = []
    for c in range(NC):
        bs = slice(c * BL, (c + 1) * BL)
        enc_c = enc[bs]                                   # [BL, S, H]
        ids = tgt[bs].T.reshape(R).astype(np.int32)       # r = t*BL + b
        h0T = np.ascontiguousarray(
            h0[bs].reshape(BL, 4, 128).transpose(2, 1, 0).reshape(128, 16)
        )
        in_maps.append({
            "emb": emb,
            "ids": np.ascontiguousarray(ids.reshape(2, 128, 1)),
            "h0T": h0T,
            "encT": np.ascontiguousarray(
                enc_c.transpose(2, 0, 1).reshape(H, BL * S).astype(bf)
            ),
            "encS": np.ascontiguousarray(
                enc_c.transpose(1, 0, 2).reshape(S, BL * H).astype(bf)
            ),
            "maskb": np.ascontiguousarray(
                np.where(mask[bs], 0.0, NEG).astype(np.float32).reshape(1, BL * S)
            ),
            "actmT": np.ascontiguousarray(np.broadcast_to(
                np.tile(
                    (np.arange(T)[:, None] < fra[bs][None, :]).astype(np.float32),
                    (1, 4),
                ).reshape(1, T * 16),
                (128, T * 16),
            )),
            "wihT": wihT,
            "whhT": whhT,
            "brow": brow,
            "bhhn4": bhhn4,
            "sel4": sel4,
            "wccT": wccT,
            "bcc": bcc4,
            "wo8": wo8i,
            "onesd": np.ones((1, 256), np.float32),
        })
    return in_maps


def assemble_output(core_outs):
    """core_outs: list of 8 arrays [R, V] bf16 (rows r = t*BL + b)."""
    out = np.empty((B, T, V), np.float32)
    for c in range(NC):
        o = np.asarray(core_outs[c]).astype(np.float32).reshape(T, BL, V)
        out[c * BL:(c + 1) * BL] = o.transpose(1, 0, 2)
    return out


def kernel(**inputs) -> np.ndarray:
    from concourse.bass_utils import run_bass_kernel_spmd
    nc = _get_program()
    in_maps = make_core_inputs(**inputs)
    res = run_bass_kernel_spmd(nc, in_maps, list(range(NC)))
    out = assemble_output([res.results[c]["out"] for c in range(NC)])
    b_out = np.asarray(inputs["b_out"], np.float32)
    if np.any(b_out):
        # Exact correction: log_softmax(l + b) = log_softmax(log_softmax(l) + b)
        x = out + b_out[None, None, :]
        m = x.max(axis=-1, keepdims=True)
        out = x - (m + np.log(np.exp(x - m).sum(axis=-1, keepdims=True)))
    return out
